# revision 21
# baseline (speedup 1.0000x reference)
"""Trainium2 (Bass/Tile) segment-sum kernel, 8-core SPMD, fp8 streaming.

Computes out[v, :] = sum over rows n with X_node[n] == v of H[n, :]
(equivalent to jax.ops.segment_sum(H, X_node, num_segments=V)).

Strategy (fp8 supergroups):
  The op is memory-bound: H is 819 MB in f32 and every algorithm must read
  it exactly once, so the only lever on DMA time is bytes/element. The
  kernel streams H as ONE fp8-e4m3 plane (1 B/elem). Plain RTNE e4m3
  quantization gives ~2.7e-2 segment-sum error (too coarse); instead the
  host quantizes with per-(segment, feature) error feedback (sigma-delta):
  rows of a segment are quantized in order with the running residual
  carried into the next row, so the SUM of the quantized rows tracks the
  exact sum to half a quantization step (~5.0e-3 relative overall; the
  harness gate is 2e-2).

  host: stable-argsort rows by segment id; split the sorted order into 8
    contiguous chunks (one per core). Rows are greedily grouped into W
    windows, each covering <=WSEG consecutive segments and <=T*128 rows
    (segments may split across windows/cores; partial sums are added on
    the host). Windows are padded so all 8 cores run ONE static SPMD
    program; 8 windows form a "supergroup" sharing one one-hot build,
    one h-load DMA, one 2-bank PSUM tile and one evacuation copy.
  device, per supergroup s: DVE builds the fp8 one-hot
    oh[p, t, v] = (iota[v] == lid[p, t]) fused over two supergroups;
    TensorE runs DoubleRow fp8 matmuls (two 128-row k-tiles per
    instruction at 0.5 cycles/row, one-hot stationary) accumulating
    window j into columns [j*128, (j+1)*128) of a [WSEG, 1024] f32 PSUM
    tile (DoubleRow dst must start at partition 0, so windows pack along
    the PSUM free dim); ActE copies PSUM->SBUF converting to bf16 (two
    supergroups pack one [WSEG, 2048] tile along columns) and issues the
    output DMA on its own HWDGE ring. NO DMA may ride GPSIMD/SWDGE: the
    SWDGE descriptor rings live on SBUF partitions whose AXI ports also
    serve SDMA engines 0 and 15, and with SWDGE active those engines
    drain their equal FIFO share of the h stream measurably slower —
    every run with the output on gpsimd ended with E0/E15 holding a
    40-120 packet backlog that serialized the last 7-15us of the stream.
  host: add the per-core window partials into the full [V, D] f32 output
    at each window's base segment.

Why WSEG=24/T=6 (vs 32/8 in v1-v2, 16/4 in v3): uniform segments of ~32
rows force T ~ WSEG/4 (window rows ~ 32*WSEG vs cap T*128, mismatch =
padding), and per MB streamed the one-hot build costs ~0.07*WSEG us on
DVE while the PSUM evacuation costs ~34.6/WSEG us on ACT (engine cost =
columns x 1 cycle regardless of populated partitions). v1/v2 (WSEG=32)
were DVE-paced at ~2.5us/MB: the profile showed every h dma_start
release tracking the one-hot cadence with mid-stream DMA bursts of
400-420 GB/s. v3 (WSEG=16) made ACT the pacer (2.2us/MB). WSEG=24 puts
DVE ~1.7 and ACT ~1.4us/MB, both under the ~2.4us/MB DMA floor, at 4.3%
padding. (v4's transposed matmul fixed ACT but made the 256x128 h slice
the stationary operand - PE busy doubled to 106us; reverted.)
"""

import numpy as np
from contextlib import ExitStack

import ml_dtypes
import concourse.tile as tile
from concourse import bacc, mybir
from concourse.bass_utils import run_bass_kernel_spmd

F32 = mybir.dt.float32
BF16 = mybir.dt.bfloat16
FP8 = mybir.dt.float8e4
NP_BF16 = ml_dtypes.bfloat16
NP_FP8 = ml_dtypes.float8_e4m3
P = 128  # partitions / tile rows
D = 128  # feature dim
WSEG = 24  # segments per window
SG = 8  # windows per supergroup
N_CORES = 8
T_CANDIDATES = (6,)  # tiles per window; even for DoubleRow
HBUFS = 12  # h prefetch depth (T*SG*D bytes/partition each)
# fp8-e4m3-exact codes for local segment ids (integers are only exact up
# to 16, then evens to 32): host maps lid -> LID_VALS[lid], device
# compares against the same list.
LID_VALS = np.array(list(range(16)) + list(range(16, 32, 2)), np.float32)[:WSEG]
assert len(LID_VALS) == WSEG and len(np.unique(LID_VALS.astype(NP_FP8))) == WSEG

LAST_RESULTS = None  # test-harness hook: BassKernelResults of the last run
_NC_CACHE = {}  # (S, T) -> compiled Bacc program


def _build_nc_cached(S: int, T: int):
    key = (S, T)
    if key not in _NC_CACHE:
        _NC_CACHE[key] = _build_nc(S, T)
    return _NC_CACHE[key]


def _build_nc(S: int, T: int):
    nc = bacc.Bacc(
        "TRN2",
        target_bir_lowering=False,
        debug=False,
        enable_asserts=False,
        num_devices=N_CORES,
    )
    FT = SG * T  # row-tiles per supergroup
    S2 = (S + 1) // 2  # output tiles (2 supergroups per tile)
    # h layout [sgroup][partition][tile][d]: per-partition runs of FT*D
    # bytes per load.
    h = nc.dram_tensor("h", [S, P, FT, D], FP8, kind="ExternalInput")
    lid = nc.dram_tensor("lid", [P, S * FT], FP8, kind="ExternalInput")
    iota = nc.dram_tensor("iota", [P, WSEG], FP8, kind="ExternalInput")
    out = nc.dram_tensor("out", [S, WSEG, SG * D], BF16, kind="ExternalOutput")

    with tile.TileContext(nc) as tc, ExitStack() as ctx:
        const = ctx.enter_context(tc.tile_pool(name="const", bufs=1))
        hpool = ctx.enter_context(tc.tile_pool(name="hw", bufs=HBUFS))
        ohpool = ctx.enter_context(tc.tile_pool(name="oh", bufs=6))
        opool = ctx.enter_context(tc.tile_pool(name="ot", bufs=4))
        psum = ctx.enter_context(tc.tile_pool(name="acc", bufs=3, space="PSUM"))
        wpsum = ctx.enter_context(tc.tile_pool(name="warm", bufs=1, space="PSUM"))

        # h prefetches go first so the SDMA engines have bulk work
        # immediately; the constants ride the ACT ring and land a couple
        # of loads in.
        def load_h(s, pool=None, tag="ht"):
            ht = (pool or hpool).tile([P, FT, D], FP8, tag=tag)
            nc.sync.dma_start(ht[:], h[s])
            return ht

        hts = {s: load_h(s) for s in range(min(5, S))}
        # the LAST supergroups' tiles are dedicated and loaded up-front:
        # a load issued at the end of the stream gets its descriptors
        # dealt to whichever SDMA rings happen to be empty (measured: one
        # engine took 116 of the final load's ~150 packets and drained
        # them serially for ~7us). Issued inside the initial flood the
        # dealing is balanced, and the stream never ends ring-empty.
        for s in range(max(5, S - 2), S):
            hts[s] = load_h(s, pool=const, tag=f"ht_tail{s}")

        iota_sb = const.tile([P, WSEG], FP8)
        nc.scalar.dma_start(iota_sb[:], iota[:])
        lid_sb = const.tile([P, S * FT], FP8)
        nc.scalar.dma_start(lid_sb[:], lid[:])

        # PE warmup: the HAM clock gate keeps the PE at 1.2 GHz until it
        # has seen ~3.4us of sustained matmul activity, and the first
        # real matmuls (~18us in) pace the h-load slot recycling — every
        # run showed a 2x DMA-rate dip at t=25-35us while the PE warmed
        # up. Burn ~6us of garbage matmuls into a scratch PSUM bank as
        # soon as iota lands (~10.5us) so the real matmuls start warm.
        warm = wpsum.tile([WSEG, WSEG], F32)
        for _ in range(96):
            nc.tensor.matmul(
                warm[:], iota_sb[:, :WSEG], iota_sb[:, :WSEG], start=True, stop=True
            )

        oh = None
        ot = None
        for s in range(S):
            ht = hts.pop(s) if s in hts else load_h(s)
            hslice = lambda t0, _h=ht: _h[:, t0 : t0 + 2, :]
            # one fused DVE op builds the one-hot tiles for TWO
            # supergroups (halves the per-instruction overhead):
            # oh[p, t, v] = (iota[p, v] == lid[p, s*FT + t])
            hh = s % 2
            if hh == 0:
                span = min(2, S - s)
                # inner pitch padded to 32: DoubleRow LDWEIGHTS requires
                # the row-group step to be a multiple of 16
                # (s3_lw_dual_fp8_restrictions); only :WSEG is written/read.
                oh = ohpool.tile([P, 2 * FT, 32], FP8, tag="oh")
                nc.vector.tensor_tensor(
                    oh[:, : span * FT, :WSEG],
                    iota_sb[:].unsqueeze(1).broadcast_to((P, span * FT, WSEG)),
                    lid_sb[:, s * FT : (s + span) * FT]
                    .unsqueeze(2)
                    .broadcast_to((P, span * FT, WSEG)),
                    mybir.AluOpType.is_equal,
                )
            # 8 windows pack one [WSEG, 8D] PSUM tile (two 2KB banks)
            # along the free dim: window j lands at columns [j*D,
            # (j+1)*D). All matmuls write base partition 0 — DoubleRow
            # matmuls with a non-zero dst base partition fail the walrus
            # ISA check.
            acc = psum.tile([WSEG, SG * D], F32)
            for j in range(SG):
                co = j * D
                for tp in range(T // 2):  # DoubleRow: two row-tiles per mm
                    t0 = j * T + 2 * tp
                    nc.tensor.matmul(
                        acc[:, co : co + D],
                        oh[:, hh * FT + t0 : hh * FT + t0 + 2, :WSEG],
                        hslice(t0),
                        start=(tp == 0),
                        stop=(tp == T // 2 - 1),
                        perf_mode=mybir.MatmulPerfMode.DoubleRow,
                    )
            # per-supergroup output tile, alternating between two
            # 32-aligned partition blocks: SDMA engine assignment is by
            # SBUF partition (4 partitions/engine; even engines serve
            # partitions 0-63, odd 64-127), so a fixed [0, 24) placement
            # put the whole output stream on 6 even engines and the h
            # stream then waited ~4us on engine 0's FIFO backlog at
            # every stream end. Even supergroups evacuate via ACT into
            # partitions [0, 24), odd ones via DVE into [64, 88) (the
            # copy shifts partitions; offsets must be 32-aligned), and
            # the store DMAs alternate engine halves. All store DMAs
            # are issued by the scalar engine (HWDGE; gpsimd/SWDGE DMA
            # poisons SDMA engines 0/15 - see module docstring).
            ot = opool.tile([P, SG * D], BF16, tag="ot")
            off = 64 * hh
            otp = ot[off : off + WSEG]
            cp = nc.scalar.copy if hh == 0 else nc.vector.tensor_copy
            if s == S - 1:
                # split the final evacuation + store so the last store
                # overlaps the last copy
                halfd = SG * D // 2
                cp(otp[:, :halfd], acc[:, :halfd])
                nc.scalar.dma_start(out[s, :, :halfd], otp[:, :halfd])
                cp(otp[:, halfd:], acc[:, halfd:])
                nc.scalar.dma_start(out[s, :, halfd:], otp[:, halfd:])
            else:
                cp(otp[:], acc[:])
                nc.scalar.dma_start(out[s], otp[:])

    nc.compile()
    return nc


def _quantize_sigma_delta(Hs: np.ndarray, sidx: np.ndarray, V: int) -> np.ndarray:
    """Quantize sorted rows Hs to fp8-e4m3 with per-(segment, feature) error
    feedback, so each segment's quantized sum tracks the exact sum to half a
    quantization step. Processes rows layer-by-layer (i-th member of every
    segment at once) to vectorize the sequential carry recurrence."""
    N = Hs.shape[0]
    starts = np.searchsorted(sidx, np.arange(V + 1))
    rank = np.arange(N) - starts[sidx]
    order2 = np.lexsort((sidx, rank))  # layer-major, segment-minor
    L = int(rank.max()) + 1
    layer_bounds = np.searchsorted(rank[order2], np.arange(L + 1))
    Q = np.empty((N, D), NP_FP8)
    carry = np.zeros((V, D), np.float32)
    for i in range(L):
        sl = order2[layer_bounds[i] : layer_bounds[i + 1]]
        segs = sidx[sl]
        x = Hs[sl] + carry[segs]
        q = x.astype(NP_FP8)
        carry[segs] = x - q.astype(np.float32)
        Q[sl] = q
    return Q


def _prepare(H: np.ndarray, X: np.ndarray, V: int):
    """Host-side sort + greedy windowing + sigma-delta fp8 + swizzle.

    Returns (in_maps, wbase[k, w] window base segments, S, T).
    """
    N, Dd = H.shape
    assert Dd == D and N % N_CORES == 0
    nloc = N // N_CORES
    X = np.ascontiguousarray(X).astype(np.int64, copy=False)
    perm = np.argsort(X, kind="stable")
    sidx = X[perm]

    def greedy(T):
        # greedy windows per core: <=T*128 rows and <=WSEG-segment span each
        cap = T * P
        bounds = []  # per core: row-rank boundaries [0, ..., nloc]
        for k in range(N_CORES):
            s = sidx[k * nloc : (k + 1) * nloc]
            b = [0]
            r = 0
            while r < nloc:
                r = min(r + cap, int(np.searchsorted(s, s[r] + WSEG, side="left")))
                b.append(r)
            bounds.append(np.asarray(b, np.int64))
        W = max(len(b) - 1 for b in bounds)
        Wp = -(-W // SG) * SG  # pad to whole supergroups
        return bounds, Wp

    best = None
    for T in T_CANDIDATES:
        bounds, Wp = greedy(T)
        if best is None or Wp * T < best[2] * best[1]:
            best = (bounds, T, Wp)
    bounds, T, Wp = best
    S = Wp // SG
    FT = SG * T

    # per-row window index / rank / local segment id
    wbase = np.full((N_CORES, Wp), V, np.int64)  # pad windows point past V
    win = np.empty(N, np.int64)
    rank = np.empty(N, np.int64)
    for k in range(N_CORES):
        b = bounds[k]
        s = sidx[k * nloc : (k + 1) * nloc]
        idx = np.arange(nloc)
        wk = np.searchsorted(b, idx, side="right") - 1
        win[k * nloc : (k + 1) * nloc] = wk
        rank[k * nloc : (k + 1) * nloc] = idx - b[wk]
        wbase[k, : len(b) - 1] = s[b[:-1]]

    k_arr = np.repeat(np.arange(N_CORES), nloc)
    lid_val = sidx - wbase[k_arr, win]
    p_arr = rank & (P - 1)
    t_arr = rank >> 7
    s_arr = win // SG
    j_arr = win % SG

    # fp8 rows, swizzled: [core][sgroup][partition][window][tile][d] so
    # each partition's DRAM run within a supergroup is contiguous
    # (SG*T*D bytes)
    Q = _quantize_sigma_delta(H[perm], sidx, V)
    rowslot = ((k_arr * S + s_arr) * P + p_arr) * FT + j_arr * T + t_arr
    hq = np.zeros((N_CORES * S * P * FT, D), NP_FP8)
    hq[rowslot] = Q
    hq = hq.reshape(N_CORES, S, P, FT, D)

    # local segment ids, mapped through the fp8-exact code list; pad = -1
    lid = np.full((N_CORES, P, Wp * T), -1.0, NP_FP8)
    lidslot = (k_arr * P + p_arr) * (Wp * T) + win * T + t_arr
    lid.reshape(-1)[lidslot] = LID_VALS[lid_val].astype(NP_FP8)

    iota = np.ascontiguousarray(np.broadcast_to(LID_VALS.astype(NP_FP8), (P, WSEG)))

    in_maps = [{"h": hq[k], "lid": lid[k], "iota": iota} for k in range(N_CORES)]
    return in_maps, wbase, S, T


def kernel(H, X_node, V, trace: bool = False) -> np.ndarray:
    global LAST_RESULTS
    H = np.asarray(H, dtype=np.float32)
    X = np.asarray(X_node)
    V = int(V)

    in_maps, wbase, S, T = _prepare(H, X, V)
    nc = _build_nc_cached(S, T)
    res = run_bass_kernel_spmd(nc, in_maps, list(range(N_CORES)), trace=trace)
    LAST_RESULTS = res

    out = np.zeros((V + WSEG, D), np.float32)
    for k in range(N_CORES):
        # out dram is [S, WSEG, SG, D] bf16: window j at column block j
        o = np.asarray(res.results[k]["out"]).reshape(S, WSEG, SG, D)
        o = o.astype(np.float32)
        for w in range(SG * S):
            b = int(wbase[k, w])
            out[b : b + WSEG] += o[w // SG, :, w % SG]
    return np.ascontiguousarray(out[:V])


# revision 23
# speedup vs baseline: 1.1365x; 1.1365x over previous
"""Trainium2 (Bass/Tile) segment-sum kernel, 8-core SPMD, fp8 streaming.

Computes out[v, :] = sum over rows n with X_node[n] == v of H[n, :]
(equivalent to jax.ops.segment_sum(H, X_node, num_segments=V)).

Strategy (fp8 supergroups):
  The op is memory-bound: H is 819 MB in f32 and every algorithm must read
  it exactly once, so the only lever on DMA time is bytes/element. The
  kernel streams H as ONE fp8-e4m3 plane (1 B/elem). Plain RTNE e4m3
  quantization gives ~2.7e-2 segment-sum error (too coarse); instead the
  host quantizes with per-(segment, feature) error feedback (sigma-delta):
  rows of a segment are quantized in order with the running residual
  carried into the next row, so the SUM of the quantized rows tracks the
  exact sum to half a quantization step (~5.0e-3 relative overall; the
  harness gate is 2e-2).

  host: stable-argsort rows by segment id; split the sorted order into 8
    contiguous chunks (one per core). Rows are greedily grouped into W
    windows, each covering <=WSEG consecutive segments and <=T*128 rows
    (segments may split across windows/cores; partial sums are added on
    the host). Windows are padded so all 8 cores run ONE static SPMD
    program; 8 windows form a "supergroup" sharing one one-hot build,
    one h-load DMA, one 2-bank PSUM tile and one evacuation copy.
  device, per supergroup s: DVE builds the fp8 one-hot
    oh[p, t, v] = (iota[v] == lid[p, t]) fused over two supergroups;
    TensorE runs DoubleRow fp8 matmuls (two 128-row k-tiles per
    instruction at 0.5 cycles/row, one-hot stationary) accumulating
    window j into columns [j*128, (j+1)*128) of a [WSEG, 1024] f32 PSUM
    tile (DoubleRow dst must start at partition 0, so windows pack along
    the PSUM free dim); ActE copies PSUM->SBUF converting to bf16 (two
    supergroups pack one [WSEG, 2048] tile along columns) and issues the
    output DMA on its own HWDGE ring. NO DMA may ride GPSIMD/SWDGE: the
    SWDGE descriptor rings live on SBUF partitions whose AXI ports also
    serve SDMA engines 0 and 15, and with SWDGE active those engines
    drain their equal FIFO share of the h stream measurably slower —
    every run with the output on gpsimd ended with E0/E15 holding a
    40-120 packet backlog that serialized the last 7-15us of the stream.
  host: add the per-core window partials into the full [V, D] f32 output
    at each window's base segment.

Why WSEG=24/T=6 (vs 32/8 in v1-v2, 16/4 in v3): uniform segments of ~32
rows force T ~ WSEG/4 (window rows ~ 32*WSEG vs cap T*128, mismatch =
padding), and per MB streamed the one-hot build costs ~0.07*WSEG us on
DVE while the PSUM evacuation costs ~34.6/WSEG us on ACT (engine cost =
columns x 1 cycle regardless of populated partitions). v1/v2 (WSEG=32)
were DVE-paced at ~2.5us/MB: the profile showed every h dma_start
release tracking the one-hot cadence with mid-stream DMA bursts of
400-420 GB/s. v3 (WSEG=16) made ACT the pacer (2.2us/MB). WSEG=24 puts
DVE ~1.7 and ACT ~1.4us/MB, both under the ~2.4us/MB DMA floor, at 4.3%
padding. (v4's transposed matmul fixed ACT but made the 256x128 h slice
the stationary operand - PE busy doubled to 106us; reverted.)
"""

import numpy as np
from contextlib import ExitStack

import ml_dtypes
import concourse.tile as tile
from concourse import bacc, mybir
from concourse.bass_utils import run_bass_kernel_spmd

F32 = mybir.dt.float32
BF16 = mybir.dt.bfloat16
FP8 = mybir.dt.float8e4
NP_BF16 = ml_dtypes.bfloat16
NP_FP8 = ml_dtypes.float8_e4m3
P = 128  # partitions / tile rows
D = 128  # feature dim
WSEG = 24  # segments per window
SG = 8  # windows per supergroup
N_CORES = 8
T_CANDIDATES = (6,)  # tiles per window; even for DoubleRow
HBUFS = 14  # h prefetch depth (T*SG*D bytes/partition each)
# fp8-e4m3-exact codes for local segment ids (integers are only exact up
# to 16, then evens to 32): host maps lid -> LID_VALS[lid], device
# compares against the same list.
LID_VALS = np.array(list(range(16)) + list(range(16, 32, 2)), np.float32)[:WSEG]
assert len(LID_VALS) == WSEG and len(np.unique(LID_VALS.astype(NP_FP8))) == WSEG

LAST_RESULTS = None  # test-harness hook: BassKernelResults of the last run
_NC_CACHE = {}  # (S, T) -> compiled Bacc program


def _build_nc_cached(S: int, T: int):
    key = (S, T)
    if key not in _NC_CACHE:
        _NC_CACHE[key] = _build_nc(S, T)
    return _NC_CACHE[key]


def _build_nc(S: int, T: int):
    nc = bacc.Bacc(
        "TRN2",
        target_bir_lowering=False,
        debug=False,
        enable_asserts=False,
        num_devices=N_CORES,
    )
    FT = SG * T  # row-tiles per supergroup
    S2 = (S + 1) // 2  # output tiles (2 supergroups per tile)
    # h layout [sgroup][partition][tile][d]: per-partition runs of FT*D
    # bytes per load.
    h = nc.dram_tensor("h", [S, P, FT, D], FP8, kind="ExternalInput")
    lid = nc.dram_tensor("lid", [P, S * FT], FP8, kind="ExternalInput")
    iota = nc.dram_tensor("iota", [P, WSEG], FP8, kind="ExternalInput")
    out2 = nc.dram_tensor("out", [S2, WSEG, 2 * SG * D], BF16, kind="ExternalOutput")

    with tile.TileContext(nc) as tc, ExitStack() as ctx:
        const = ctx.enter_context(tc.tile_pool(name="const", bufs=1))
        hpool = ctx.enter_context(tc.tile_pool(name="hw", bufs=HBUFS))
        ohpool = ctx.enter_context(tc.tile_pool(name="oh", bufs=6))
        opool = ctx.enter_context(tc.tile_pool(name="ot", bufs=4))
        psum = ctx.enter_context(tc.tile_pool(name="acc", bufs=4, space="PSUM"))

        # h prefetches go first so the SDMA engines have bulk work
        # immediately; the constants ride the ACT ring and land a couple
        # of loads in.
        def load_h(s, pool=None, tag="ht"):
            ht = (pool or hpool).tile([P, FT, D], FP8, tag=tag)
            nc.sync.dma_start(ht[:], h[s])
            return ht

        hts = {s: load_h(s) for s in range(min(5, S))}
        # the LAST supergroups' tiles are dedicated and loaded up-front:
        # a load issued at the end of the stream gets its descriptors
        # dealt to whichever SDMA rings happen to be empty (measured: one
        # engine took 116 of the final load's ~150 packets and drained
        # them serially for ~7us). Issued inside the initial flood the
        # dealing is balanced, and the stream never ends ring-empty.
        for s in range(max(5, S - 3), S):
            hts[s] = load_h(s, pool=const, tag=f"ht_tail{s}")

        iota_sb = const.tile([P, WSEG], FP8)
        nc.scalar.dma_start(iota_sb[:], iota[:])
        lid_sb = const.tile([P, S * FT], FP8)
        # first piece covers supergroups 0-1 (96B/partition, lands right
        # behind iota) so the one-hot pipeline ramps ~3us earlier; the
        # rest follows in one load
        nc.scalar.dma_start(lid_sb[:, : 2 * FT], lid[:, : 2 * FT])
        nc.scalar.dma_start(lid_sb[:, 2 * FT :], lid[:, 2 * FT :])

        oh = None
        ot = None
        for s in range(S):
            ht = hts.pop(s) if s in hts else load_h(s)
            hslice = lambda t0, _h=ht: _h[:, t0 : t0 + 2, :]
            # one fused DVE op builds the one-hot tiles for TWO
            # supergroups (halves the per-instruction overhead):
            # oh[p, t, v] = (iota[p, v] == lid[p, s*FT + t])
            hh = s % 2
            if hh == 0:
                span = min(2, S - s)
                # inner pitch padded to 32: DoubleRow LDWEIGHTS requires
                # the row-group step to be a multiple of 16
                # (s3_lw_dual_fp8_restrictions); only :WSEG is written/read.
                oh = ohpool.tile([P, 2 * FT, 32], FP8, tag="oh")
                nc.vector.tensor_tensor(
                    oh[:, : span * FT, :WSEG],
                    iota_sb[:].unsqueeze(1).broadcast_to((P, span * FT, WSEG)),
                    lid_sb[:, s * FT : (s + span) * FT]
                    .unsqueeze(2)
                    .broadcast_to((P, span * FT, WSEG)),
                    mybir.AluOpType.is_equal,
                )
            # 8 windows pack one [WSEG, 8D] PSUM tile (two 2KB banks)
            # along the free dim: window j lands at columns [j*D,
            # (j+1)*D). All matmuls write base partition 0 — DoubleRow
            # matmuls with a non-zero dst base partition fail the walrus
            # ISA check.
            acc = psum.tile([WSEG, SG * D], F32)
            for j in range(SG):
                co = j * D
                for tp in range(T // 2):  # DoubleRow: two row-tiles per mm
                    t0 = j * T + 2 * tp
                    nc.tensor.matmul(
                        acc[:, co : co + D],
                        oh[:, hh * FT + t0 : hh * FT + t0 + 2, :WSEG],
                        hslice(t0),
                        start=(tp == 0),
                        stop=(tp == T // 2 - 1),
                        perf_mode=mybir.MatmulPerfMode.DoubleRow,
                    )
            # two supergroups share one [24, 2048] output tile (along
            # columns) at partitions [0, 24), evacuated by ACT and stored
            # by one scalar-issued HWDGE DMA per pair (gpsimd/SWDGE DMA
            # poisons SDMA engines 0/15 - see module docstring).
            # Partition-rotating the tile across pairs was tried to
            # spread the output over both SDMA engine halves: an ACT copy
            # with a shifted dst partition block compiles but writes
            # garbage on HW (rel err 9e-2), and a DVE copy (whose shift
            # IS correct) costs 1.55us/SG with no fast uop, pushing DVE
            # past the DMA pace. Fixed block + ACT it is.
            if hh == 0:
                ot = opool.tile([P, 2 * SG * D], BF16, tag="ot")
            otp = ot[:WSEG]
            co = hh * SG * D
            if s == S - 1:
                # split the final evacuation + store so the last store
                # overlaps the last copy; the first store also covers the
                # pair partner's columns [0, co) when S is even
                halfd = SG * D // 2
                nc.scalar.copy(otp[:, co : co + halfd], acc[:, :halfd])
                nc.scalar.dma_start(out2[s // 2, :, : co + halfd], otp[:, : co + halfd])
                nc.scalar.copy(otp[:, co + halfd :], acc[:, halfd:])
                nc.scalar.dma_start(out2[s // 2, :, co + halfd :], otp[:, co + halfd :])
            else:
                nc.scalar.copy(otp[:, co : co + SG * D], acc[:])
                if hh == 1:
                    nc.scalar.dma_start(out2[s // 2], otp[:])

    nc.compile()
    return nc


def _quantize_sigma_delta(Hs: np.ndarray, sidx: np.ndarray, V: int) -> np.ndarray:
    """Quantize sorted rows Hs to fp8-e4m3 with per-(segment, feature) error
    feedback, so each segment's quantized sum tracks the exact sum to half a
    quantization step. Processes rows layer-by-layer (i-th member of every
    segment at once) to vectorize the sequential carry recurrence."""
    N = Hs.shape[0]
    starts = np.searchsorted(sidx, np.arange(V + 1))
    rank = np.arange(N) - starts[sidx]
    order2 = np.lexsort((sidx, rank))  # layer-major, segment-minor
    L = int(rank.max()) + 1
    layer_bounds = np.searchsorted(rank[order2], np.arange(L + 1))
    Q = np.empty((N, D), NP_FP8)
    carry = np.zeros((V, D), np.float32)
    for i in range(L):
        sl = order2[layer_bounds[i] : layer_bounds[i + 1]]
        segs = sidx[sl]
        x = Hs[sl] + carry[segs]
        q = x.astype(NP_FP8)
        carry[segs] = x - q.astype(np.float32)
        Q[sl] = q
    return Q


def _prepare(H: np.ndarray, X: np.ndarray, V: int):
    """Host-side sort + greedy windowing + sigma-delta fp8 + swizzle.

    Returns (in_maps, wbase[k, w] window base segments, S, T).
    """
    N, Dd = H.shape
    assert Dd == D and N % N_CORES == 0
    nloc = N // N_CORES
    X = np.ascontiguousarray(X).astype(np.int64, copy=False)
    perm = np.argsort(X, kind="stable")
    sidx = X[perm]

    def greedy(T):
        # greedy windows per core: <=T*128 rows and <=WSEG-segment span each
        cap = T * P
        bounds = []  # per core: row-rank boundaries [0, ..., nloc]
        for k in range(N_CORES):
            s = sidx[k * nloc : (k + 1) * nloc]
            b = [0]
            r = 0
            while r < nloc:
                r = min(r + cap, int(np.searchsorted(s, s[r] + WSEG, side="left")))
                b.append(r)
            bounds.append(np.asarray(b, np.int64))
        W = max(len(b) - 1 for b in bounds)
        Wp = -(-W // SG) * SG  # pad to whole supergroups
        return bounds, Wp

    best = None
    for T in T_CANDIDATES:
        bounds, Wp = greedy(T)
        if best is None or Wp * T < best[2] * best[1]:
            best = (bounds, T, Wp)
    bounds, T, Wp = best
    S = Wp // SG
    FT = SG * T

    # per-row window index / rank / local segment id
    wbase = np.full((N_CORES, Wp), V, np.int64)  # pad windows point past V
    win = np.empty(N, np.int64)
    rank = np.empty(N, np.int64)
    for k in range(N_CORES):
        b = bounds[k]
        s = sidx[k * nloc : (k + 1) * nloc]
        idx = np.arange(nloc)
        wk = np.searchsorted(b, idx, side="right") - 1
        win[k * nloc : (k + 1) * nloc] = wk
        rank[k * nloc : (k + 1) * nloc] = idx - b[wk]
        wbase[k, : len(b) - 1] = s[b[:-1]]

    k_arr = np.repeat(np.arange(N_CORES), nloc)
    lid_val = sidx - wbase[k_arr, win]
    p_arr = rank & (P - 1)
    t_arr = rank >> 7
    s_arr = win // SG
    j_arr = win % SG

    # fp8 rows, swizzled: [core][sgroup][partition][window][tile][d] so
    # each partition's DRAM run within a supergroup is contiguous
    # (SG*T*D bytes)
    Q = _quantize_sigma_delta(H[perm], sidx, V)
    rowslot = ((k_arr * S + s_arr) * P + p_arr) * FT + j_arr * T + t_arr
    hq = np.zeros((N_CORES * S * P * FT, D), NP_FP8)
    hq[rowslot] = Q
    hq = hq.reshape(N_CORES, S, P, FT, D)

    # local segment ids, mapped through the fp8-exact code list; pad = -1
    lid = np.full((N_CORES, P, Wp * T), -1.0, NP_FP8)
    lidslot = (k_arr * P + p_arr) * (Wp * T) + win * T + t_arr
    lid.reshape(-1)[lidslot] = LID_VALS[lid_val].astype(NP_FP8)

    iota = np.ascontiguousarray(np.broadcast_to(LID_VALS.astype(NP_FP8), (P, WSEG)))

    in_maps = [{"h": hq[k], "lid": lid[k], "iota": iota} for k in range(N_CORES)]
    return in_maps, wbase, S, T


def kernel(H, X_node, V, trace: bool = False) -> np.ndarray:
    global LAST_RESULTS
    H = np.asarray(H, dtype=np.float32)
    X = np.asarray(X_node)
    V = int(V)

    in_maps, wbase, S, T = _prepare(H, X, V)
    nc = _build_nc_cached(S, T)
    res = run_bass_kernel_spmd(nc, in_maps, list(range(N_CORES)), trace=trace)
    LAST_RESULTS = res

    out = np.zeros((V + WSEG, D), np.float32)
    for k in range(N_CORES):
        # out dram is [S2, WSEG, 2, SG, D] bf16: supergroup s in column
        # half s%2, window j at column block j
        S2 = (S + 1) // 2
        o = np.asarray(res.results[k]["out"]).reshape(S2, WSEG, 2, SG, D)
        o = o.astype(np.float32)
        for w in range(SG * S):
            b = int(wbase[k, w])
            s = w // SG
            out[b : b + WSEG] += o[s // 2, :, s % 2, w % SG]
    return np.ascontiguousarray(out[:V])
